# revision 11
# baseline (speedup 1.0000x reference)
"""Trainium2 Bass kernel for the 3-axis finite-difference nonzero-count stencil.

Computes, for x of shape [1, 512, 512, 64] (f32):
    y = (x[w+1] != x[w-1]) + (x[h+1] != x[h-1]) + (x[d+1] != x[d-1])
with zero padding outside the grid (each term is 1.0 where the central
difference is nonzero, else 0.0 -- fd != 0  <=>  the two shifted operands
differ, including signed-zero cases, so a direct compare is exact).

Distribution: the D axis (512) is sharded 8 ways (64 rows per core); the
1-row D halo is delivered host-side in each core's input slice, so no
collectives are needed.

Per-core layout: 128 partitions = 32 h-blocks x 4 d-groups (p = hb*4 + g).
Each partition owns a [16d, 16h, 64w] output block and holds its haloed
[18, 18, 66] input window in the free dimension (the host pre-gathers the
overlapping windows, zero-padded at the grid borders), so all three stencil
compares are free-axis tensor_tensor(not_equal) ops:
  - DVE: W-compare and D-compare (f32)
  - GpSimd: H-compare (keeps DVE under the HBM roofline)
  - PE: 3-way sum of the bf16 compare maps via identity matmuls
    accumulating in PSUM (f32)
  - ScalarE: PSUM -> SBUF evacuation (f32)
  - DMA: HWDGE only, big contiguous transfers
"""

import sys

sys.path.insert(0, "/opt/trn_rl_repo")

import numpy as np
import ml_dtypes

import concourse.bass as bass
import concourse.bacc as bacc
import concourse.tile as tile
from concourse import mybir
from concourse.bass_utils import run_bass_kernel_spmd

D, H, W = 512, 512, 64
NCORES = 8
DS = D // NCORES  # 64 d-rows per core

f32 = mybir.dt.float32
bf16 = mybir.dt.bfloat16
NE = mybir.AluOpType.not_equal

# d-chunk ranges of the 18 d-slots for pipelined input DMAs; each output
# subtile (4 d-rows) reads a 6-slot window touching at most 2 chunks.
DCHUNKS = [(0, 6), (6, 10), (10, 14), (14, 18)]


def _kernel_body(tc, x_ext, id_ext, y_ext):
    nc = tc.nc
    with (
        tc.tile_pool(name="big", bufs=1) as big,
        tc.tile_pool(name="cmp", bufs=3) as cmp_pool,
        tc.tile_pool(name="yout", bufs=2) as ypool,
        tc.tile_pool(name="psum", bufs=8, space="PSUM") as pp,
    ):
        ident = big.tile([128, 128], bf16, tag="ident")
        nc.sync.dma_start(ident[:], id_ext[:])

        # whole per-core haloed input resident in SBUF: 85.5 KiB/partition;
        # partition p holds its own [18, 18, 66] pre-gathered window
        xt = big.tile([128, 18, 18, 66], f32, tag="xt")
        for dlo, dhi in DCHUNKS:
            nc.sync.dma_start(xt[:, dlo:dhi], x_ext[:, dlo:dhi])

        for s in range(4):
            ds = 1 + 4 * s  # output d-slots ds..ds+4 of the 18-slot window
            cw = cmp_pool.tile([128, 4, 16, 64], bf16, tag="cw")
            nc.vector.tensor_tensor(
                cw[:],
                xt[:, ds : ds + 4, 1:17, 2:66],
                xt[:, ds : ds + 4, 1:17, 0:64],
                NE,
            )
            cd = cmp_pool.tile([128, 4, 16, 64], bf16, tag="cd")
            nc.vector.tensor_tensor(
                cd[:],
                xt[:, ds + 1 : ds + 5, 1:17, 1:65],
                xt[:, ds - 1 : ds + 3, 1:17, 1:65],
                NE,
            )
            # GpSimd's Q7 tensor_tensor ucode rejects comparison ALU ops, so
            # the H-compare also runs on DVE (still under the HBM roofline).
            ch = cmp_pool.tile([128, 4, 16, 64], bf16, tag="ch")
            nc.vector.tensor_tensor(
                ch[:],
                xt[:, ds : ds + 4, 2:18, 1:65],
                xt[:, ds : ds + 4, 0:16, 1:65],
                NE,
            )

            yt = ypool.tile([128, 4096], f32, tag="y")
            cwf = cw[:].rearrange("p a b c -> p (a b c)")
            cdf = cd[:].rearrange("p a b c -> p (a b c)")
            chf = ch[:].rearrange("p a b c -> p (a b c)")
            for j in range(8):
                sl = slice(512 * j, 512 * (j + 1))
                ps = pp.tile([128, 512], f32, tag="ps")
                nc.tensor.matmul(ps[:], ident[:], cwf[:, sl], start=True, stop=False)
                nc.tensor.matmul(ps[:], ident[:], cdf[:, sl], start=False, stop=False)
                nc.tensor.matmul(ps[:], ident[:], chf[:, sl], start=False, stop=True)
                nc.scalar.copy(yt[:, sl], ps[:])

            nc.sync.dma_start(y_ext[s], yt[:])


_NC_CACHE = {}


def _build(n_iters=1):
    if n_iters not in _NC_CACHE:
        nc = bacc.Bacc()
        x_ext = nc.declare_dram_parameter(
            "x", [128, 18, 18, 66], f32, isOutput=False
        )
        id_ext = nc.declare_dram_parameter("ident", [128, 128], bf16, isOutput=False)
        y_ext = nc.declare_dram_parameter("y", [4, 128, 4096], f32, isOutput=True)
        with tile.TileContext(nc) as tc:
            for _ in range(n_iters):
                _kernel_body(tc, x_ext, id_ext, y_ext)
        nc.compile()
        _NC_CACHE[n_iters] = nc
    return _NC_CACHE[n_iters]


def _make_in_maps(x):
    xs = np.ascontiguousarray(x.reshape(D, H, W).astype(np.float32, copy=False))
    ident = np.eye(128, dtype=ml_dtypes.bfloat16)
    in_maps = []
    for c in range(NCORES):
        xp = np.zeros((DS + 2, H + 2, W + 2), np.float32)
        dlo, dhi = DS * c - 1, DS * c + DS + 1
        slo, shi = max(dlo, 0), min(dhi, D)
        xp[slo - dlo : shi - dlo, 1 : H + 1, 1 : W + 1] = xs[slo:shi]
        # gather the 128 per-partition haloed windows: p = hb*4 + g holds
        # xp[16g : 16g+18, 16hb : 16hb+18, :]
        sd, sh, sw = xp.strides
        win = np.lib.stride_tricks.as_strided(
            xp,
            shape=(32, 4, 18, 18, W + 2),
            strides=(16 * sh, 16 * sd, sd, sh, sw),
        )
        in_maps.append(
            {"x": win.reshape(128, 18, 18, W + 2).copy(), "ident": ident}
        )
    return in_maps


def _assemble(res):
    # per-core y: [4, 128, 4096] -> [s, hb, g, di, hi, w] -> d = 16g + 4s + di
    ys = []
    for c in range(NCORES):
        yb = np.asarray(res[c]["y"]).reshape(4, 32, 4, 4, 16, W)
        y = yb.transpose(2, 0, 3, 1, 4, 5).reshape(DS, H, W)
        ys.append(y)
    return np.concatenate(ys, axis=0).reshape(1, D, H, W)


def kernel(x):
    x = np.asarray(x)
    assert x.shape == (1, D, H, W), x.shape
    nc = _build()
    res = run_bass_kernel_spmd(nc, _make_in_maps(x), list(range(NCORES))).results
    return _assemble(res).astype(np.float32, copy=False)
